# revision 1
# baseline (speedup 1.0000x reference)
"""ConvolvedAttention (sliding-window causal attention, W=33) on 8 TRN2 NeuronCores.

Sharding: sequence L=8192 split 8 ways (1024 tokens/core), data-parallel over
cores. Host passes each core its query shard plus key/value shards with a
32-token halo on the left; projections are replicated. Each core runs a fused
Bass/Tile kernel: qkv projections -> banded scores (k-major, query-aligned
128-key supers) -> masked softmax -> AV -> out-projection. Host folds in the
output biases and reassembles.
"""

import numpy as np

# ---- problem constants (hardcoded per contract) ----
L, N, E = 8192, 2, 256
H, HD = 8, 32
WHALF = 32            # window//2 ; attended span = 33 (past only)
NCORES = 8
T = L // NCORES       # 1024 tokens per core
TL = 128 + T          # local tokens per batch entry: 96 pad + 32 halo + 1024
NEG = -1e9
NSUP = 9              # supers 0..8 ; super 0 = pad+halo block

# wpack column layout (fp32 cols per partition)
_WQ = 0               # 4 tiles [128,128]  (ki*2+ko)
_WK = 512
_WV = 1024            # 2 tiles [128,256]  (ki)
_WO = 1536            # 2 tiles [128,256]  (g = E_in chunk)
_BQ = 2048            # 2 cols  (ko)
_BK = 2050            # 2 cols
_BD = 2052            # [8,256] block-diag indicator (2 groups of 128 cols)
_I128 = 2308          # [128,128] identity
_MMAIN = 2436         # [128,320] band mask, 2 heads tiled (additive 0/-1e9)
_M0 = 2756            # [128,64] super-0 mask (pad+halo), 2 heads tiled
_ONES32 = 2820        # [128,32] all-ones (S-sum lhsT)
_WPCOLS = 2852

_STATE = {}


def _sup_w(s):
    return 32 if s == 0 else (128 if s == NSUP - 1 else 160)


def _build_program():
    import os
    level = int(os.environ.get("KBUILD_LEVEL", "9"))
    import concourse.bacc as bacc
    import concourse.tile as tile
    import concourse.mybir as mybir
    from contextlib import ExitStack

    f32 = mybir.dt.float32
    AF = mybir.ActivationFunctionType

    nc = bacc.Bacc("TRN2", target_bir_lowering=False, debug=False)
    xq_d = nc.declare_dram_parameter("xq", [2, 128, 2 * T], f32, isOutput=False)
    xk_d = nc.declare_dram_parameter("xk", [2, 128, 2 * TL], f32, isOutput=False)
    xv_d = nc.declare_dram_parameter("xv", [2, 128, 2 * TL], f32, isOutput=False)
    wp_d = nc.declare_dram_parameter("wpack", [128, _WPCOLS], f32, isOutput=False)
    out_d = nc.declare_dram_parameter("out", [2, 8, 128, 256], f32, isOutput=True)

    ones_col = nc.const_aps.tensor(1.0, (128, 1))

    with ExitStack() as stk:
        tc = stk.enter_context(tile.TileContext(nc))
        sb = stk.enter_context(tc.tile_pool(name="sb", bufs=1))
        sb_probs = stk.enter_context(tc.tile_pool(name="probs", bufs=2))
        sb_tr = stk.enter_context(tc.tile_pool(name="tr", bufs=3))

        # ---- load inputs ----
        wp = sb.tile([128, _WPCOLS], f32, tag="wp")
        nc.sync.dma_start(wp[:], wp_d[:])
        xq = []
        xk = []
        xv = []
        for ki in range(2):
            t_q = sb.tile([128, 2 * T], f32, tag=f"xq{ki}", name=f"xq{ki}")
            nc.sync.dma_start(t_q[:], xq_d[ki])
            xq.append(t_q)
            t_k = sb.tile([128, 2 * TL], f32, tag=f"xk{ki}", name=f"xk{ki}")
            nc.sync.dma_start(t_k[:], xk_d[ki])
            xk.append(t_k)
            t_v = sb.tile([128, 2 * TL], f32, tag=f"xv{ki}", name=f"xv{ki}")
            nc.sync.dma_start(t_v[:], xv_d[ki])
            xv.append(t_v)

        q_sb = [sb.tile([128, 2 * T], f32, tag=f"q{ko}", name=f"q{ko}") for ko in range(2)]
        k_sb = [sb.tile([128, 2 * TL], f32, tag=f"k{ko}", name=f"k{ko}") for ko in range(2)]
        v_sb = [sb.tile([128, 256], f32, tag=f"v{b}", name=f"v{b}") for b in range(2 * NSUP)]

        # ---- phase 1: projections ----
        with tc.tile_pool(name="pp", bufs=3, space="PSUM") as pp:
            # q / k projections: out [E_out chunk, tokens]
            for ko in range(2):
                bq_ap = wp[:, _BQ + ko : _BQ + ko + 1]
                bk_ap = wp[:, _BK + ko : _BK + ko + 1]
                for g0 in range(0, 2 * T, 512):
                    ps = pp.tile([128, 512], f32, tag="pq", name="pq")
                    for ki in range(2):
                        nc.tensor.matmul(
                            ps[:],
                            wp[:, _WQ + (ki * 2 + ko) * 128 : _WQ + (ki * 2 + ko + 1) * 128],
                            xq[ki][:, g0 : g0 + 512],
                            start=(ki == 0),
                            stop=(ki == 1),
                        )
                    nc.scalar.activation(
                        q_sb[ko][:, g0 : g0 + 512], ps[:], AF.Identity, bias=bq_ap
                    )
                for g0 in range(0, 2 * TL, 512):
                    w = min(512, 2 * TL - g0)
                    ps = pp.tile([128, 512], f32, tag="pq", name="pq")
                    for ki in range(2):
                        nc.tensor.matmul(
                            ps[:, :w],
                            wp[:, _WK + (ki * 2 + ko) * 128 : _WK + (ki * 2 + ko + 1) * 128],
                            xk[ki][:, g0 : g0 + w],
                            start=(ki == 0),
                            stop=(ki == 1),
                        )
                    nc.scalar.activation(
                        k_sb[ko][:, g0 : g0 + w], ps[:, :w], AF.Identity, bias=bk_ap
                    )
            # v projection: out [tokens, E_out]
            for b in range(2 * NSUP):
                ps = pp.tile([128, 256], f32, tag="pv", name="pv")
                for ki in range(2):
                    nc.tensor.matmul(
                        ps[:],
                        xv[ki][:, b * 128 : (b + 1) * 128],
                        wp[:, _WV + ki * 256 : _WV + (ki + 1) * 256],
                        start=(ki == 0),
                        stop=(ki == 1),
                    )
                nc.vector.tensor_copy(v_sb[b][:], ps[:])

        # ---- phase 2: attention ----
        i128 = wp[:, _I128 : _I128 + 128]
        mmain = wp[:, _MMAIN : _MMAIN + 320].rearrange("p (t w) -> p t w", t=2)
        m0 = wp[:, _M0 : _M0 + 64]

        with (
            tc.tile_pool(name="psc", bufs=4, space="PSUM") as psc,
            tc.tile_pool(name="pav", bufs=2, space="PSUM") as pav,
            tc.tile_pool(name="pms", bufs=2, space="PSUM") as pms,
        ):
            for n in range(2):
                probs = {}
                for s in range(NSUP):
                    w = _sup_w(s)
                    qs = 0 if s == 0 else 128 * (s - 1)
                    pr = sb_probs.tile([128, 8 * 160], f32, tag="probs", name="probs")
                    probs[s] = pr
                    if level < 2:
                        continue
                    pr_r = pr[:, : 8 * w].rearrange("p (a b) -> p a b", a=8)
                    for j in range(4):
                        # bank j: heads j and j+4, both at row group 32j
                        sc = psc.tile([128, 2 * 160], f32, tag="sc", name="sc")
                        if s == 0:
                            nc.tensor.matmul(
                                sc[:, : 2 * w], i128, m0,
                                start=True, stop=False, skip_group_check=True,
                            )
                        else:
                            nc.tensor.matmul(
                                sc[:, : 2 * w], i128, mmain[:, :, :w],
                                start=True, stop=False, skip_group_check=True,
                            )
                        for hb in range(2):
                            h = j + 4 * hb
                            ch, hr = hb, 32 * j
                            nc.tensor.matmul(
                                sc[:, hb * w : (hb + 1) * w],
                                k_sb[ch][hr : hr + 32, n * TL + 128 * s : n * TL + 128 * s + 128],
                                q_sb[ch][hr : hr + 32, n * T + qs : n * T + qs + w],
                                start=False, stop=(hb == 1),
                                tile_position=(hr, 0), skip_group_check=True,
                            )
                        nc.scalar.activation(
                            pr_r[:, j::4, :], sc[:, : 2 * w], AF.Exp,
                        )
                    if s == 0 or level < 3:
                        continue
                    # finalize query block a = s-1 (queries 128a .. 128a+128)
                    a = s - 1
                    wp_prev = _sup_w(s - 1)
                    wc = min(w, 128)
                    pcur, pprev = probs[s], probs[s - 1]
                    sps = pms.tile([128, 256], f32, tag="ms", name="ms")
                    ones32 = wp[:, _ONES32 : _ONES32 + 32]
                    for h in range(8):
                        hp, hc = 32 * (h % 4), 128 * (h // 4)
                        nc.tensor.matmul(
                            sps[hp : hp + 32, hc : hc + wc], ones32,
                            pcur[:, h * w : h * w + wc],
                            start=True, stop=False, skip_group_check=True,
                            tile_position=(0, hp),
                        )
                        nc.tensor.matmul(
                            sps[hp : hp + 32, hc : hc + 32], ones32,
                            pprev[:, h * wp_prev + wp_prev - 32 : h * wp_prev + wp_prev],
                            start=False, stop=True, skip_group_check=True,
                            tile_position=(0, hp),
                        )
                    s_sb = sb_tr.tile([128, 256], f32, tag="ssb", name="ssb")
                    nc.vector.tensor_copy(s_sb[:], sps[:])
                    s_r = sb_tr.tile([128, 256], f32, tag="sr", name="sr")
                    nc.vector.reciprocal_approx_fast(out=s_r[:], in_=s_sb[:])
                    if level < 4:
                        o_sb = sb_tr.tile([128, 256], f32, tag="osb", name="osb")
                        nc.vector.tensor_copy(o_sb[:], s_r[:])
                        nc.sync.dma_start(out_d[n, a], o_sb[:])
                        continue
                    avn = []
                    for g in range(2):
                        av = pav.tile([128, 128], f32, tag="av", name="av")
                        for hb in range(4):
                            h = 4 * g + hb
                            hr = 32 * hb
                            nc.tensor.matmul(
                                av[hr : hr + 32, :wc],
                                v_sb[NSUP * n + s][:, 32 * h : 32 * h + 32],
                                pcur[:, h * w : h * w + wc],
                                start=True, stop=False,
                                tile_position=(0, hr), skip_group_check=True,
                            )
                            nc.tensor.matmul(
                                av[hr : hr + 32, :32],
                                v_sb[NSUP * n + s - 1][:, 32 * h : 32 * h + 32],
                                pprev[:, h * wp_prev + wp_prev - 32 : h * wp_prev + wp_prev],
                                start=False, stop=True,
                                tile_position=(0, hr), skip_group_check=True,
                            )
                        t_avn = sb_tr.tile([128, 128], f32, tag="avn", name="avn")
                        if level >= 5:
                            nc.vector.tensor_mul(t_avn[:], av[:], s_r[:, 128 * g : 128 * (g + 1)])
                        else:
                            nc.vector.tensor_copy(t_avn[:], av[:])
                        avn.append(t_avn)
                    op = pms.tile([128, 256], f32, tag="ms", name="ms")
                    for g in range(2):
                        nc.tensor.matmul(
                            op[:], avn[g][:],
                            wp[:, _WO + g * 256 : _WO + (g + 1) * 256],
                            start=(g == 0), stop=(g == 1),
                        )
                    o_sb = sb_tr.tile([128, 256], f32, tag="osb", name="osb")
                    nc.scalar.copy(o_sb[:], op[:])
                    nc.sync.dma_start(out_d[n, a], o_sb[:])
                    del probs[s - 1]
                if level < 3:
                    for a in range(8):
                        o_sb = sb_tr.tile([128, 256], f32, tag="osb", name="osb")
                        if level >= 2:
                            nc.vector.tensor_copy(o_sb[:], probs[a][:, :256])
                        else:
                            nc.vector.tensor_copy(o_sb[:], v_sb[a][:])
                        nc.sync.dma_start(out_d[n, a], o_sb[:])
    nc.compile()
    return nc


def _host_prep(query, key, value, in_proj_w, in_proj_b, out_proj_w, out_proj_b):
    """Build per-core input maps + the host-side output bias vector."""
    s = 1.0 / np.sqrt(HD)
    wq = (in_proj_w[:E] * s).astype(np.float32)
    wk = in_proj_w[E : 2 * E].astype(np.float32)
    wv = in_proj_w[2 * E :].astype(np.float32)
    bq = (in_proj_b[:E] * s).astype(np.float32)
    bk = in_proj_b[E : 2 * E].astype(np.float32)
    bv = in_proj_b[2 * E :].astype(np.float32)
    wo = out_proj_w.astype(np.float32)

    wpack_base = np.zeros((128, _WPCOLS), np.float32)
    wqT, wkT = wq.T.copy(), wk.T.copy()   # [E_in, E_out]
    for ki in range(2):
        for ko in range(2):
            wpack_base[:, _WQ + (ki * 2 + ko) * 128 : _WQ + (ki * 2 + ko + 1) * 128] = \
                wqT[ki * 128 : (ki + 1) * 128, ko * 128 : (ko + 1) * 128]
            wpack_base[:, _WK + (ki * 2 + ko) * 128 : _WK + (ki * 2 + ko + 1) * 128] = \
                wkT[ki * 128 : (ki + 1) * 128, ko * 128 : (ko + 1) * 128]
        wpack_base[:, _WV + ki * 256 : _WV + (ki + 1) * 256] = \
            wv.T[ki * 128 : (ki + 1) * 128, :]
        wpack_base[:, _WO + ki * 256 : _WO + (ki + 1) * 256] = \
            wo.T[ki * 128 : (ki + 1) * 128, :]
    for ko in range(2):
        wpack_base[:, _BQ + ko] = bq[ko * 128 : (ko + 1) * 128]
        wpack_base[:, _BK + ko] = bk[ko * 128 : (ko + 1) * 128]
    # block-diag indicator [8, 256]: row k, col 128g+p -> 1 iff k == 4g + p//32
    for g in range(2):
        for hh in range(4):
            wpack_base[4 * g + hh, _BD + 128 * g + 32 * hh : _BD + 128 * g + 32 * (hh + 1)] = 1.0
    wpack_base[:128, _I128 : _I128 + 128] = np.eye(128, dtype=np.float32)
    wpack_base[:, _ONES32 : _ONES32 + 32] = 1.0
    # band mask [128, 2x160]: valid iff 0 <= c - rho <= WHALF
    rho = np.arange(128)[:, None]
    c = np.arange(160)[None, :]
    band = np.where((c - rho >= 0) & (c - rho <= WHALF), 0.0, NEG).astype(np.float32)
    wpack_base[:, _MMAIN : _MMAIN + 160] = band
    wpack_base[:, _MMAIN + 160 : _MMAIN + 320] = band

    # super-0 mask [128, 2x32]: rows 0..96 pad -> NEG ; rows 96..128 halo
    m0 = np.full((128, 64), NEG, np.float32)
    i = np.arange(32)[:, None]
    qt = np.arange(32)[None, :]
    tri = np.where(qt <= i, 0.0, NEG).astype(np.float32)
    m0[96:128, 0:32] = tri
    m0[96:128, 32:64] = tri

    qf = np.ascontiguousarray(query.transpose(2, 1, 0).astype(np.float32))  # [E, N, L]
    kf = np.ascontiguousarray(key.transpose(2, 1, 0).astype(np.float32))
    vf = np.ascontiguousarray(value.transpose(2, 1, 0).astype(np.float32))

    in_maps = []
    for cidx in range(NCORES):
        l0 = cidx * T
        xq = qf[:, :, l0 : l0 + T].reshape(2, 128, N * T)
        xk = np.zeros((2, 128, N, TL), np.float32)
        xv = np.zeros((2, 128, N, TL), np.float32)
        kfc = kf.reshape(2, 128, N, L)
        vfc = vf.reshape(2, 128, N, L)
        xk[:, :, :, 128:] = kfc[:, :, :, l0 : l0 + T]
        xv[:, :, :, 128:] = vfc[:, :, :, l0 : l0 + T]
        if cidx > 0:
            xk[:, :, :, 96:128] = kfc[:, :, :, l0 - 32 : l0]
            xv[:, :, :, 96:128] = vfc[:, :, :, l0 - 32 : l0]
        wpack = wpack_base.copy()
        if cidx == 0:
            wpack[:, _M0 : _M0 + 64] = NEG
        else:
            wpack[:, _M0 : _M0 + 64] = m0
        in_maps.append(
            {
                "xq": np.ascontiguousarray(xq),
                "xk": np.ascontiguousarray(xk.reshape(2, 128, N * TL)),
                "xv": np.ascontiguousarray(xv.reshape(2, 128, N * TL)),
                "wpack": wpack,
            }
        )
    add_vec = (out_proj_b + bv @ wo.T).astype(np.float32)
    return in_maps, add_vec


def _get_state():
    if "nc" not in _STATE:
        _STATE["nc"] = _build_program()
    return _STATE["nc"]


def kernel(query, key, value, in_proj_w, in_proj_b, out_proj_w, out_proj_b,
           collect_intermediates=0, _trace=False):
    from concourse.bass_utils import run_bass_kernel_spmd

    nc = _get_state()
    in_maps, add_vec = _host_prep(
        np.asarray(query), np.asarray(key), np.asarray(value),
        np.asarray(in_proj_w), np.asarray(in_proj_b),
        np.asarray(out_proj_w), np.asarray(out_proj_b),
    )
    res = run_bass_kernel_spmd(nc, in_maps, list(range(NCORES)), trace=_trace)
    out = np.empty((L, N, E), np.float32)
    for cidx in range(NCORES):
        dev = res.results[cidx]["out"]  # [2, 8, 128, 256]
        shard = dev.transpose(1, 2, 0, 3).reshape(T, N, E)
        out[cidx * T : (cidx + 1) * T] = shard
    out += add_vec
    if _trace:
        _STATE["last_exec_ns"] = res.exec_time_ns
        _STATE["last_res"] = res
    return out



# revision 2
# speedup vs baseline: 2.6338x; 2.6338x over previous
"""ConvolvedAttention (sliding-window causal attention, W=33) on 8 TRN2 NeuronCores.

Sharding: sequence L=8192 split 8 ways (1024 tokens/core), data-parallel over
cores. Host passes each core its query shard plus key/value shards with a
32-token halo on the left; projections are replicated. Each core runs a fused
Bass/Tile kernel in bf16: qkv projections -> banded scores (k-major,
query-aligned supers, row-tiled 4-way concurrent) -> exp (one batched ACT per
super) -> 0/1 band mask multiply on DVE -> S-sum / AV (col-tiled) -> out
projection. Host folds in output biases and reassembles. The K-projection
bias is dropped entirely: it contributes a per-query constant to every score,
which cancels in softmax.
"""

import numpy as np
import ml_dtypes

# ---- problem constants (hardcoded per contract) ----
L, N, E = 8192, 2, 256
H, HD = 8, 32
WHALF = 32            # window//2 ; attended span = 33 (past only)
NCORES = 8
T = L // NCORES       # 1024 tokens per core
TL = 128 + T          # local K/V tokens per batch entry: 96 pad + 32 halo + 1024
NSUP = 9              # supers 0..8 ; super 0 = pad+halo block
BF = ml_dtypes.bfloat16

# wpack column layout (bf16 cols per partition)
_WQ = 0               # 4 tiles [128,128]  (ki*2+ko)
_WK = 512
_WV = 1024            # 2 tiles [128,256]  (ki)
_WO = 1536            # 2 tiles [128,256]  (g = E_in chunk)
_ONES32 = 2048        # [128,32] all-ones (S-sum lhsT)
_BQ = 2080            # 2 cols  (ko)
_BREP = 2082          # [128, 8*160] band mask 0/1, replicated per head slot
_B0REP = 2082 + 8 * 160   # [128, 8*32] super-0 mask 0/1
_WPCOLS = _B0REP + 8 * 32

# head h -> slot index in scores/probs layouts.  Chosen so that the four
# concurrently-streaming row-tiled score matmuls (j = h%4) land in four
# different PSUM banks (slot*256 : slots 2j and 2j+1 -> bank j).
_SL = [(h % 4) * 2 + h // 4 for h in range(H)]

_STATE = {}


def _sup_w(s):
    return 32 if s == 0 else (128 if s == NSUP - 1 else 160)


def _build_program():
    import concourse.bacc as bacc
    import concourse.tile as tile
    import concourse.mybir as mybir
    from contextlib import ExitStack

    f32 = mybir.dt.float32
    bf16 = mybir.dt.bfloat16
    AF = mybir.ActivationFunctionType

    nc = bacc.Bacc("TRN2", target_bir_lowering=False, debug=False)
    xq_d = nc.declare_dram_parameter("xq", [2, 128, 2 * T], bf16, isOutput=False)
    xk_d = nc.declare_dram_parameter("xk", [2, 128, 2 * TL], bf16, isOutput=False)
    xv_d = nc.declare_dram_parameter("xv", [2, 128, 2 * TL], bf16, isOutput=False)
    wp_d = nc.declare_dram_parameter("wpack", [128, _WPCOLS], bf16, isOutput=False)
    out_d = nc.declare_dram_parameter("out", [2, 8, 128, 256], bf16, isOutput=True)

    with ExitStack() as stk:
        tc = stk.enter_context(tile.TileContext(nc))
        sb = stk.enter_context(tc.tile_pool(name="sb", bufs=1))
        sb_probs = stk.enter_context(tc.tile_pool(name="probs", bufs=3))
        sb_tr = stk.enter_context(tc.tile_pool(name="tr", bufs=3))

        # ---- load inputs ----
        wp = sb.tile([128, _WPCOLS], bf16, tag="wp")
        nc.sync.dma_start(wp[:], wp_d[:])
        xq = []
        xk = []
        xv = []
        for ki in range(2):
            t_q = sb.tile([128, 2 * T], bf16, tag=f"xq{ki}", name=f"xq{ki}")
            nc.sync.dma_start(t_q[:], xq_d[ki])
            xq.append(t_q)
            t_k = sb.tile([128, 2 * TL], bf16, tag=f"xk{ki}", name=f"xk{ki}")
            nc.sync.dma_start(t_k[:], xk_d[ki])
            xk.append(t_k)
            t_v = sb.tile([128, 2 * TL], bf16, tag=f"xv{ki}", name=f"xv{ki}")
            nc.sync.dma_start(t_v[:], xv_d[ki])
            xv.append(t_v)

        q_sb = [sb.tile([128, 2 * T], bf16, tag=f"q{ko}", name=f"q{ko}") for ko in range(2)]
        k_sb = [sb.tile([128, 2 * TL], bf16, tag=f"k{ko}", name=f"k{ko}") for ko in range(2)]
        v_sb = [sb.tile([128, 256], bf16, tag=f"v{b}", name=f"v{b}") for b in range(2 * NSUP)]

        # ---- phase 1: projections ----
        with tc.tile_pool(name="pp", bufs=3, space="PSUM") as pp:
            for ko in range(2):
                bq_ap = wp[:, _BQ + ko : _BQ + ko + 1]
                for g0 in range(0, 2 * T, 512):
                    ps = pp.tile([128, 512], f32, tag="pq", name="pq")
                    for ki in range(2):
                        nc.tensor.matmul(
                            ps[:],
                            wp[:, _WQ + (ki * 2 + ko) * 128 : _WQ + (ki * 2 + ko + 1) * 128],
                            xq[ki][:, g0 : g0 + 512],
                            start=(ki == 0),
                            stop=(ki == 1),
                        )
                    nc.scalar.activation(
                        q_sb[ko][:, g0 : g0 + 512], ps[:], AF.Identity, bias=bq_ap
                    )
                for g0 in range(0, 2 * TL, 512):
                    w = min(512, 2 * TL - g0)
                    ps = pp.tile([128, 512], f32, tag="pq", name="pq")
                    for ki in range(2):
                        nc.tensor.matmul(
                            ps[:, :w],
                            wp[:, _WK + (ki * 2 + ko) * 128 : _WK + (ki * 2 + ko + 1) * 128],
                            xk[ki][:, g0 : g0 + w],
                            start=(ki == 0),
                            stop=(ki == 1),
                        )
                    nc.vector.tensor_copy(k_sb[ko][:, g0 : g0 + w], ps[:, :w])
            # v projection: out [tokens, E_out]
            for b in range(2 * NSUP):
                ps = pp.tile([128, 256], f32, tag="pv", name="pv")
                for ki in range(2):
                    nc.tensor.matmul(
                        ps[:],
                        xv[ki][:, b * 128 : (b + 1) * 128],
                        wp[:, _WV + ki * 256 : _WV + (ki + 1) * 256],
                        start=(ki == 0),
                        stop=(ki == 1),
                    )
                nc.vector.tensor_copy(v_sb[b][:], ps[:])

        # ---- phase 2: attention ----
        brep = wp[:, _BREP : _BREP + 8 * 160].rearrange("p (a w) -> p a w", a=8)
        b0rep = wp[:, _B0REP : _B0REP + 8 * 32].rearrange("p (a w) -> p a w", a=8)

        with (
            tc.tile_pool(name="psc", bufs=1, space="PSUM") as psc,
            tc.tile_pool(name="pav", bufs=2, space="PSUM") as pav,
            tc.tile_pool(name="pms", bufs=2, space="PSUM") as pms,
        ):
            for n in range(2):
                probs = {}
                for s in range(NSUP):
                    w = _sup_w(s)
                    qs = 0 if s == 0 else 128 * (s - 1)
                    # scores: head h -> psum cols SL[h]*256, row-tiled over j=h%4
                    scp = psc.tile([128, 2048], f32, tag="sc", name="sc")
                    for hb in range(2):
                        for j in range(4):
                            h = j + 4 * hb
                            sl = _SL[h]
                            nc.tensor.matmul(
                                scp[:, sl * 256 : sl * 256 + w],
                                k_sb[hb][32 * j : 32 * j + 32,
                                         n * TL + 128 * s : n * TL + 128 * s + 128],
                                q_sb[hb][32 * j : 32 * j + 32,
                                         n * T + qs : n * T + qs + w],
                                start=True, stop=True,
                                tile_position=(32 * j, 0), skip_group_check=True,
                            )
                    # exp: one batched activation over all 8 head slots
                    pr = sb_probs.tile([128, 8 * 160], bf16, tag="probs", name="probs")
                    probs[s] = pr
                    scp_v = scp[:].rearrange("p (a c) -> p a c", a=8)[:, :, :w]
                    pr_v = pr[:].rearrange("p (a c) -> p a c", a=8)[:, :, :w]
                    nc.scalar.activation(pr_v, scp_v, AF.Exp)
                    # band mask multiply (0/1), in place on DVE
                    msk = b0rep if s == 0 else brep[:, :, :w]
                    nc.vector.tensor_mul(pr_v, pr_v, msk)
                    if s == 0:
                        continue
                    # finalize query block a = s-1
                    a = s - 1
                    wp_prev = _sup_w(s - 1)
                    wc = min(w, 128)
                    pcur, pprev = probs[s], probs[s - 1]
                    ones32 = wp[:, _ONES32 : _ONES32 + 32]
                    sps = pms.tile([128, 256], f32, tag="ms", name="ms")
                    for h in range(8):
                        hp, hc = 32 * (h % 4), 128 * (h // 4)
                        c_cur = _SL[h] * 160
                        c_prev = _SL[h] * 160 + wp_prev - 32
                        nc.tensor.matmul(
                            sps[hp : hp + 32, hc : hc + wc], ones32,
                            pcur[:, c_cur : c_cur + wc],
                            start=True, stop=False, skip_group_check=True,
                            tile_position=(0, hp),
                        )
                        nc.tensor.matmul(
                            sps[hp : hp + 32, hc : hc + 32], ones32,
                            pprev[:, c_prev : c_prev + 32],
                            start=False, stop=True, skip_group_check=True,
                            tile_position=(0, hp),
                        )
                    s_r = sb_tr.tile([128, 256], f32, tag="sr", name="sr")
                    nc.vector.reciprocal_approx_fast(out=s_r[:], in_=sps[:])
                    avn = []
                    for g in range(2):
                        av = pav.tile([128, 128], f32, tag="av", name="av")
                        for hb in range(4):
                            h = 4 * g + hb
                            hr = 32 * hb
                            c_cur = _SL[h] * 160
                            c_prev = _SL[h] * 160 + wp_prev - 32
                            nc.tensor.matmul(
                                av[hr : hr + 32, :wc],
                                v_sb[NSUP * n + s][:, 32 * h : 32 * h + 32],
                                pcur[:, c_cur : c_cur + wc],
                                start=True, stop=False,
                                tile_position=(0, hr), skip_group_check=True,
                            )
                            nc.tensor.matmul(
                                av[hr : hr + 32, :32],
                                v_sb[NSUP * n + s - 1][:, 32 * h : 32 * h + 32],
                                pprev[:, c_prev : c_prev + 32],
                                start=False, stop=True,
                                tile_position=(0, hr), skip_group_check=True,
                            )
                        t_avn = sb_tr.tile([128, 128], bf16, tag="avn", name="avn")
                        nc.vector.tensor_mul(t_avn[:], av[:], s_r[:, 128 * g : 128 * (g + 1)])
                        avn.append(t_avn)
                    op = pms.tile([128, 256], f32, tag="ms", name="ms")
                    for g in range(2):
                        nc.tensor.matmul(
                            op[:], avn[g][:],
                            wp[:, _WO + g * 256 : _WO + (g + 1) * 256],
                            start=(g == 0), stop=(g == 1),
                        )
                    o_sb = sb_tr.tile([128, 256], bf16, tag="osb", name="osb")
                    nc.scalar.copy(o_sb[:], op[:])
                    nc.sync.dma_start(out_d[n, a], o_sb[:])
                    del probs[s - 1]
    nc.compile()
    return nc


def _host_prep(query, key, value, in_proj_w, in_proj_b, out_proj_w, out_proj_b):
    """Build per-core input maps + the host-side output bias vector."""
    s = 1.0 / np.sqrt(HD)
    wq = (in_proj_w[:E] * s).astype(np.float32)
    wk = in_proj_w[E : 2 * E].astype(np.float32)
    wv = in_proj_w[2 * E :].astype(np.float32)
    bq = (in_proj_b[:E] * s).astype(np.float32)
    bv = in_proj_b[2 * E :].astype(np.float32)
    wo = out_proj_w.astype(np.float32)

    wpack_base = np.zeros((128, _WPCOLS), np.float32)
    wqT, wkT = wq.T.copy(), wk.T.copy()   # [E_in, E_out]
    for ki in range(2):
        for ko in range(2):
            wpack_base[:, _WQ + (ki * 2 + ko) * 128 : _WQ + (ki * 2 + ko + 1) * 128] = \
                wqT[ki * 128 : (ki + 1) * 128, ko * 128 : (ko + 1) * 128]
            wpack_base[:, _WK + (ki * 2 + ko) * 128 : _WK + (ki * 2 + ko + 1) * 128] = \
                wkT[ki * 128 : (ki + 1) * 128, ko * 128 : (ko + 1) * 128]
        wpack_base[:, _WV + ki * 256 : _WV + (ki + 1) * 256] = \
            wv.T[ki * 128 : (ki + 1) * 128, :]
        wpack_base[:, _WO + ki * 256 : _WO + (ki + 1) * 256] = \
            wo.T[ki * 128 : (ki + 1) * 128, :]
    wpack_base[:, _ONES32 : _ONES32 + 32] = 1.0
    for ko in range(2):
        wpack_base[:, _BQ + ko] = bq[ko * 128 : (ko + 1) * 128]
    # band mask 0/1 [128, 160]: valid iff 0 <= c - r <= WHALF, replicated per slot
    rho = np.arange(128)[:, None]
    c = np.arange(160)[None, :]
    band01 = ((c - rho >= 0) & (c - rho <= WHALF)).astype(np.float32)
    for a in range(8):
        wpack_base[:, _BREP + a * 160 : _BREP + (a + 1) * 160] = band01

    # super-0 mask 0/1 [128, 32]: rows 0..96 pad -> 0 ; rows 96..128 halo tri
    m0 = np.zeros((128, 32), np.float32)
    i = np.arange(32)[:, None]
    qt = np.arange(32)[None, :]
    m0[96:128, :] = (qt <= i).astype(np.float32)

    qf = np.ascontiguousarray(query.transpose(2, 1, 0).astype(np.float32))  # [E, N, L]
    kf = np.ascontiguousarray(key.transpose(2, 1, 0).astype(np.float32))
    vf = np.ascontiguousarray(value.transpose(2, 1, 0).astype(np.float32))

    in_maps = []
    for cidx in range(NCORES):
        l0 = cidx * T
        xq = qf[:, :, l0 : l0 + T].reshape(2, 128, N * T)
        xk = np.zeros((2, 128, N, TL), np.float32)
        xv = np.zeros((2, 128, N, TL), np.float32)
        kfc = kf.reshape(2, 128, N, L)
        vfc = vf.reshape(2, 128, N, L)
        xk[:, :, :, 128:] = kfc[:, :, :, l0 : l0 + T]
        xv[:, :, :, 128:] = vfc[:, :, :, l0 : l0 + T]
        if cidx > 0:
            xk[:, :, :, 96:128] = kfc[:, :, :, l0 - 32 : l0]
            xv[:, :, :, 96:128] = vfc[:, :, :, l0 - 32 : l0]
        wpack = wpack_base.copy()
        if cidx > 0:
            for a in range(8):
                wpack[:, _B0REP + a * 32 : _B0REP + (a + 1) * 32] = m0
        in_maps.append(
            {
                "xq": np.ascontiguousarray(xq).astype(BF),
                "xk": np.ascontiguousarray(xk.reshape(2, 128, N * TL)).astype(BF),
                "xv": np.ascontiguousarray(xv.reshape(2, 128, N * TL)).astype(BF),
                "wpack": wpack.astype(BF),
            }
        )
    add_vec = (out_proj_b + bv @ wo.T).astype(np.float32)
    return in_maps, add_vec


def _get_state():
    if "nc" not in _STATE:
        _STATE["nc"] = _build_program()
    return _STATE["nc"]


def kernel(query, key, value, in_proj_w, in_proj_b, out_proj_w, out_proj_b,
           collect_intermediates=0, _trace=False):
    from concourse.bass_utils import run_bass_kernel_spmd

    nc = _get_state()
    in_maps, add_vec = _host_prep(
        np.asarray(query), np.asarray(key), np.asarray(value),
        np.asarray(in_proj_w), np.asarray(in_proj_b),
        np.asarray(out_proj_w), np.asarray(out_proj_b),
    )
    res = run_bass_kernel_spmd(nc, in_maps, list(range(NCORES)), trace=_trace)
    out = np.empty((L, N, E), np.float32)
    for cidx in range(NCORES):
        dev = np.asarray(res.results[cidx]["out"], dtype=np.float32)  # [2,8,128,256]
        shard = dev.transpose(1, 2, 0, 3).reshape(T, N, E)
        out[cidx * T : (cidx + 1) * T] = shard
    out += add_vec
    if _trace:
        _STATE["last_exec_ns"] = res.exec_time_ns
        _STATE["last_res"] = res
    return out
